# revision 13
# baseline (speedup 1.0000x reference)
"""KronyMLP Trainium2 kernel — rank-32 Kronecker factorization, fp16, DMA-bound.

Math (per the reference):
    kr1 = kron(c_fc_1 [1536,32], c_fc_2 [1,12])     -> [1536, 384]
    kr2 = kron(c_proj_1 [32,1536], c_proj_2 [12,1]) -> [384, 1536]
    out = gelu(x @ kr1) @ kr2                        x: [16, 4096, 1536] f32

Key identity: kron structure makes both big matmuls rank-32:
    y  = x @ c_fc_1                    [T, 32]   (the only large contraction)
    h  = y expand-by c_fc_2            [T, 384]  h[t, j*12+l] = y[t,j]*B[l]
    g  = gelu(h)
    z  = g contract-by c_proj_2        [T, 32]   z[t,j] = sum_l g[t,j*12+l]*B2[l]
    out= z @ c_proj_1                  [T, 1536]
This cuts PE FLOPs ~12x vs the dense kr1/kr2 form; the kernel becomes
HBM-bound. fp16 I/O halves DMA traffic (tolerance is 2e-2). Output values
are ~1e-5 (subnormal in fp16), so a 2^8 scale is folded into the
contract-stage weights and divided out (exactly) on the host.

Per core (data-parallel over 8 cores, 8192 tokens each):
  - host pre-transposes x to [128, 12, 8192] fp16 (d = p*12 + c), so the
    device needs NO transposes: MM1 contracts d on partitions directly.
  - expand / contract stages are tiny matmuls against host-built
    kron-expansion matrices; gelu runs on ScalarE out of PSUM.
  - MM2 uses z^T token-blocks as stationary so the output lands in natural
    [token, d] layout for straight DMA out.
"""

import os
import numpy as np

B, S, D = 16, 4096, 1536
R = 32            # rank (c_fc_1 cols / c_proj_1 rows)
L = 12            # kron factor (c_fc_2 cols / c_proj_2 rows)
H = R * L         # 384 hidden
N_CORES = 8
T = (B // N_CORES) * S   # 8192 tokens per core
P = 128
DC = D // P       # 12 d chunks
HC = H // P       # 3 hidden chunks
ST = 1024         # tokens per supertile (one x DMA)
G = 512           # tokens per matmul group
OUT_SCALE = 256.0  # folded into M2 weights; divided out on host (exact)

_BUILT = {}


def _build(dt_name, T=T):
    import concourse.bacc as bacc
    import concourse.mybir as mybir
    from concourse.bass import ts
    from concourse.tile import TileContext

    f32 = mybir.dt.float32
    f16 = {"f16": mybir.dt.float16, "bf16": mybir.dt.bfloat16}[dt_name]
    AF = mybir.ActivationFunctionType
    # CoreSim doesn't implement Gelu; allow substituting Copy for sim runs.
    gelu_fn = AF.Copy if os.environ.get("KRONY_SIM_GELU") == "copy" else AF.Gelu

    n_st = T // ST            # 8 supertiles
    n_g = ST // G             # 2 groups per supertile
    n_b = G // P              # 4 token-blocks per group
    NO = D // 512             # 3 output column chunks

    nc = bacc.Bacc(None, target_bir_lowering=False, debug=False)
    xT_d = nc.declare_dram_parameter("xT", [P, DC, T], f16, isOutput=False)
    a_d = nc.declare_dram_parameter("afc", [P, DC, R], f16, isOutput=False)
    m1_d = nc.declare_dram_parameter("m1t", [R, HC, P], f16, isOutput=False)
    m2_d = nc.declare_dram_parameter("m2t", [P, HC, R], f16, isOutput=False)
    w2_d = nc.declare_dram_parameter("w2", [R, D], f16, isOutput=False)
    out_d = nc.declare_dram_parameter("out", [T, D], f16, isOutput=True)

    n_groups = T // G

    with TileContext(nc) as tc:
        with (
            tc.tile_pool(name="const", bufs=1) as cpool,
            tc.tile_pool(name="xin", bufs=6) as xpool,
            tc.tile_pool(name="ysb", bufs=2) as ypool_sb,
            tc.tile_pool(name="gh", bufs=2) as ghpool,
            tc.tile_pool(name="zsb", bufs=2) as zpool_sb,
            tc.tile_pool(name="outp", bufs=4) as opool,
            tc.tile_pool(name="ps_y", bufs=2, space="PSUM") as psy,
            tc.tile_pool(name="ps_h", bufs=2, space="PSUM") as psh,
            tc.tile_pool(name="ps_z", bufs=2, space="PSUM") as psz,
            tc.tile_pool(name="ps_o", bufs=2, space="PSUM") as pso,
        ):
            # a_sb first (MM1's stationary), then the first x tile halves, and
            # only then the later-needed weights — so PE starts ~6us sooner.
            a_sb = cpool.tile([P, DC, R], f16)
            nc.sync.dma_start(out=a_sb[:], in_=a_d[:, :, :])
            xt0 = xpool.tile([P, DC, G], f16, tag="xt")
            nc.sync.dma_start(out=xt0[:, 0:6, :], in_=xT_d[:, 0:6, ts(0, G)])
            nc.sync.dma_start(out=xt0[:, 6:12, :], in_=xT_d[:, 6:12, ts(0, G)])
            m1_sb = cpool.tile([R, HC, P], f16)
            nc.sync.dma_start(out=m1_sb[:], in_=m1_d[:, :, :])
            m2_sb = cpool.tile([P, HC, R], f16)
            nc.sync.dma_start(out=m2_sb[:], in_=m2_d[:, :, :])
            w2_sb = cpool.tile([R, D], f16)
            nc.sync.dma_start(out=w2_sb[:], in_=w2_d[:, :])

            # process groups in pairs, chunk-major, so each stationary weight
            # load is immediately reused by the second group of the pair —
            # _dedup_ldweights then drops the redundant LDWEIGHTS.
            for gp in range(n_groups // 2):
                gA, gB = 2 * gp, 2 * gp + 1
                xts = []
                for g in (gA, gB):
                    if g == 0:
                        xts.append(xt0)
                        continue
                    xt = xpool.tile([P, DC, G], f16, tag="xt")
                    nc.sync.dma_start(
                        out=xt[:, 0:6, :], in_=xT_d[:, 0:6, ts(g, G)]
                    )
                    nc.sync.dma_start(
                        out=xt[:, 6:12, :], in_=xT_d[:, 6:12, ts(g, G)]
                    )
                    xts.append(xt)
                # ---- MM1: y^T[32, G] = sum_c A_c^T @ x^T_c (pairwise) ----
                y_pss = []
                for g2 in range(2):
                    y_pss.append(psy.tile([R, G], f32, tag="y", name=f"yps{gp}_{g2}"))
                for c in range(DC):
                    for g2 in range(2):
                        nc.tensor.matmul(
                            y_pss[g2][:],
                            lhsT=a_sb[:, c, :],
                            rhs=xts[g2][:, c, :],
                            start=(c == 0),
                            stop=(c == DC - 1),
                        )
                y_sbs = []
                for g2 in range(2):
                    y_sb = ypool_sb.tile([R, G], f16, tag="ysb")
                    nc.vector.tensor_copy(out=y_sb[:], in_=y_pss[g2][:])
                    y_sbs.append(y_sb)
                # ---- expand (kron rows) + gelu, pairwise per chunk ----
                ghs = [
                    ghpool.tile([P, HC, G], f16, tag="gh", name=f"gh{gp}_{i}")
                    for i in range(2)
                ]
                for m in range(HC):
                    for g2 in range(2):
                        h_ps = psh.tile([P, G], f32)
                        nc.tensor.matmul(
                            h_ps[:], lhsT=m1_sb[:, m, :], rhs=y_sbs[g2][:],
                            start=True, stop=True,
                        )
                        nc.scalar.activation(
                            out=ghs[g2][:, m, :], in_=h_ps[:], func=gelu_fn,
                        )
                # ---- contract l: z^T[32, G], pairwise per chunk ----
                z_pss = [psz.tile([R, G], f32, tag="z", name=f"zps{gp}_{i}") for i in range(2)]
                for m in range(HC):
                    for g2 in range(2):
                        nc.tensor.matmul(
                            z_pss[g2][:], lhsT=m2_sb[:, m, :],
                            rhs=ghs[g2][:, m, :],
                            start=(m == 0), stop=(m == HC - 1),
                        )
                z_sbs = []
                for g2 in range(2):
                    z_sb = zpool_sb.tile([R, G], f16, tag="zsb")
                    nc.vector.tensor_copy(out=z_sb[:], in_=z_pss[g2][:])
                    z_sbs.append(z_sb)
                # ---- MM2: out[128t, D] = z_blk @ W2, natural layout ----
                for g2 in range(2):
                    g = 2 * gp + g2
                    osb = opool.tile([P, n_b, D], f16, tag="osb")
                    for b in range(n_b):
                        blk = g * n_b + b
                        for n in range(NO):
                            o_ps = pso.tile([P, 512], f32)
                            nc.tensor.matmul(
                                o_ps[:],
                                lhsT=z_sbs[g2][:, ts(b, P)],
                                rhs=w2_sb[:, ts(n, 512)],
                                start=True, stop=True,
                            )
                            # balance PSUM->SBUF casts across DVE and ACT
                            to_dve = (n == 0) or (n == 2 and blk % 8 < 5)
                            if to_dve:
                                nc.vector.tensor_copy(
                                    out=osb[:, b, ts(n, 512)], in_=o_ps[:],
                                )
                            else:
                                nc.scalar.activation(
                                    out=osb[:, b, ts(n, 512)], in_=o_ps[:],
                                    func=AF.Copy,
                                )
                        # output DMA per block on the SWDGE (gpsimd) ring so
                        # it drains in parallel with the input HWDGE ring
                        nc.gpsimd.dma_start(
                            out=out_d[ts(blk, P), :], in_=osb[:, b, :],
                        )
    nc.finalize()
    _dedup_ldweights(nc, mybir)
    return nc


def _dedup_ldweights(nc, mybir):
    """Remove InstLdweights whose stationary AP + tile params equal the
    immediately preceding tensor-engine weight load (with only matmuls in
    between). The PE array still holds those weights, so the reload is pure
    overhead (~90ns each, serialized with the matmul stream on TRN2).
    Dependencies of a removed load are migrated to the following matmul."""
    removed = 0
    for f in nc.m.functions:
        for blk in f.blocks:
            insts = list(blk.instructions)
            keep = []
            last_ldw_key = None
            pending = None  # ldw queued for possible removal
            for inst in insts:
                if isinstance(inst, mybir.InstLdweights):
                    hw = getattr(inst, "has_wait", None)
                    has_sync = bool(hw() if callable(hw) else hw)
                    key = (
                        str(inst.ins[0]),
                        str(getattr(inst, "tile_position", None)),
                        str(getattr(inst, "tile_size", None)),
                        str(getattr(inst, "perf_mode", None)),
                    )
                    if key == last_ldw_key and not has_sync:
                        pending = inst  # drop it; deps go to next matmul
                        removed += 1
                        continue
                    last_ldw_key = key
                    keep.append(inst)
                else:
                    # next inst (matmul or anything else): migrate the
                    # removed load's bookkeeping deps, keep the inst
                    if pending is not None:
                        inst.add_sync_dependencies_from(
                            pending.sync_dependency_set_copy()
                        )
                        inst.add_nosync_dependencies_from(
                            pending.nosync_dependency_set_copy()
                        )
                        pending = None
                    keep.append(inst)
            if pending is not None:
                keep.append(pending)
                removed -= 1
            if len(keep) != len(insts):
                blk.instructions = keep
    if os.environ.get("KRONY_DEBUG"):
        print(f"_dedup_ldweights: removed {removed}")
    return removed


def get_nc(dt_name=None):
    if dt_name is None:
        dt_name = os.environ.get("KRONY_DT", "f16")
    if dt_name not in _BUILT:
        _BUILT[dt_name] = _build(dt_name)
    return _BUILT[dt_name]


def _np_dt(dt_name):
    if dt_name == "f16":
        return np.float16
    import ml_dtypes
    return ml_dtypes.bfloat16


def _host_weights(c_fc_1, c_fc_2, c_proj_1, c_proj_2, np_dt):
    A = np.asarray(c_fc_1, np.float32)        # [1536, 32]
    Bv = np.asarray(c_fc_2, np.float32).reshape(L)     # [12]
    W2 = np.asarray(c_proj_1, np.float32)     # [32, 1536]
    B2 = np.asarray(c_proj_2, np.float32).reshape(L)   # [12]

    # d = p*12 + c  ->  a_dev[p, c, j] = A[p*12+c, j]
    a_dev = A.reshape(P, DC, R).astype(np_dt)
    # hidden index ph = j*12 + l ; chunk m holds ph in [m*128, (m+1)*128)
    ph = np.arange(H)
    j_of = ph // L
    l_of = ph % L
    # M1[ph, j] = B[l]*(j == j_of) ; lhsT for expand = M1^T [32, 384]
    m1t = np.zeros((R, H), np.float32)
    m1t[j_of, ph] = Bv[l_of]
    m1t_dev = m1t.reshape(R, HC, P).astype(np_dt)
    # M2[j, ph] = B2[l]*(j == j_of) (scaled); lhsT for contract = M2^T [384, 32]
    m2t = np.zeros((H, R), np.float32)
    m2t[ph, j_of] = B2[l_of] * OUT_SCALE
    m2t_dev = m2t.reshape(HC, P, R).transpose(1, 0, 2).copy()
    m2t_dev = np.ascontiguousarray(m2t_dev).astype(np_dt)
    w2_dev = W2.astype(np_dt)                 # [32, 1536]
    return a_dev, m1t_dev, m2t_dev, w2_dev


def _host_x(x, np_dt):
    """x [B, S, D] f32 -> per-core transposed [P, DC, T] fp16 (d = p*12+c)."""
    xf = np.asarray(x).reshape(N_CORES, T, P, DC).astype(np_dt)
    # [core, t, p, c] -> [core, p, c, t]
    return [np.ascontiguousarray(xf[i].transpose(1, 2, 0)) for i in range(N_CORES)]


def run_sharded(x, c_fc_1, c_fc_2, c_proj_1, c_proj_2, trace=False, tmpdir=None):
    from concourse.bass_utils import run_bass_kernel_spmd

    dt_name = os.environ.get("KRONY_DT", "f16")
    np_dt = _np_dt(dt_name)
    a_dev, m1t_dev, m2t_dev, w2_dev = _host_weights(
        c_fc_1, c_fc_2, c_proj_1, c_proj_2, np_dt
    )
    x_shards = _host_x(x, np_dt)
    in_maps = [
        {
            "xT": x_shards[i],
            "afc": a_dev,
            "m1t": m1t_dev,
            "m2t": m2t_dev,
            "w2": w2_dev,
        }
        for i in range(N_CORES)
    ]
    nc = get_nc(dt_name)
    res = run_bass_kernel_spmd(
        nc, in_maps, list(range(N_CORES)), trace=trace, tmpdir=tmpdir
    )
    outs = [res.results[i]["out"].astype(np.float32) for i in range(N_CORES)]
    full = np.concatenate(outs, axis=0) * np.float32(1.0 / OUT_SCALE)
    return full.reshape(B, S, D), res


def kernel(x, c_fc_1, c_fc_2, c_proj_1, c_proj_2):
    out, _ = run_sharded(x, c_fc_1, c_fc_2, c_proj_1, c_proj_2)
    return out.astype(np.float32)


# revision 15
# speedup vs baseline: 1.1369x; 1.1369x over previous
"""KronyMLP Trainium2 kernel — rank-32 Kronecker factorization, fp16, DMA-bound.

Math (per the reference):
    kr1 = kron(c_fc_1 [1536,32], c_fc_2 [1,12])     -> [1536, 384]
    kr2 = kron(c_proj_1 [32,1536], c_proj_2 [12,1]) -> [384, 1536]
    out = gelu(x @ kr1) @ kr2                        x: [16, 4096, 1536] f32

Key identity: kron structure makes both big matmuls rank-32:
    y  = x @ c_fc_1                    [T, 32]   (the only large contraction)
    h  = y expand-by c_fc_2            [T, 384]  h[t, j*12+l] = y[t,j]*B[l]
    g  = gelu(h)
    z  = g contract-by c_proj_2        [T, 32]   z[t,j] = sum_l g[t,j*12+l]*B2[l]
    out= z @ c_proj_1                  [T, 1536]
This cuts PE FLOPs ~12x vs the dense kr1/kr2 form; the kernel becomes
HBM-bound. fp16 I/O halves DMA traffic (tolerance is 2e-2). Output values
are ~1e-5 (subnormal in fp16), so a 2^8 scale is folded into the
contract-stage weights and divided out (exactly) on the host.

Per core (data-parallel over 8 cores, 8192 tokens each):
  - host pre-transposes x to [128, 12, 8192] fp16 (d = p*12 + c), so the
    device needs NO transposes: MM1 contracts d on partitions directly.
  - expand / contract stages are tiny matmuls against host-built
    kron-expansion matrices; gelu runs on ScalarE out of PSUM.
  - MM2 uses z^T token-blocks as stationary so the output lands in natural
    [token, d] layout for straight DMA out.
"""

import os
import numpy as np

B, S, D = 16, 4096, 1536
R = 32            # rank (c_fc_1 cols / c_proj_1 rows)
L = 12            # kron factor (c_fc_2 cols / c_proj_2 rows)
H = R * L         # 384 hidden
N_CORES = 8
T = (B // N_CORES) * S   # 8192 tokens per core
P = 128
DC = D // P       # 12 d chunks
HC = H // P       # 3 hidden chunks
ST = 1024         # tokens per supertile (one x DMA)
G = 512           # tokens per matmul group
OUT_SCALE = 256.0  # folded into M2 weights; divided out on host (exact)

_BUILT = {}


def _build(dt_name, T=T):
    import concourse.bacc as bacc
    import concourse.mybir as mybir
    from concourse.bass import ts
    from concourse.tile import TileContext

    f32 = mybir.dt.float32
    f16 = {"f16": mybir.dt.float16, "bf16": mybir.dt.bfloat16}[dt_name]
    AF = mybir.ActivationFunctionType
    # CoreSim doesn't implement Gelu; allow substituting Copy for sim runs.
    gelu_fn = AF.Copy if os.environ.get("KRONY_SIM_GELU") == "copy" else AF.Gelu

    n_st = T // ST            # 8 supertiles
    n_g = ST // G             # 2 groups per supertile
    n_b = G // P              # 4 token-blocks per group
    NO = D // 512             # 3 output column chunks

    nc = bacc.Bacc(None, target_bir_lowering=False, debug=False)
    xT_d = nc.declare_dram_parameter("xT", [P, DC, T], f16, isOutput=False)
    a_d = nc.declare_dram_parameter("afc", [P, DC, R], f16, isOutput=False)
    m1_d = nc.declare_dram_parameter("m1t", [R, HC, P], f16, isOutput=False)
    m2_d = nc.declare_dram_parameter("m2t", [P, HC, R], f16, isOutput=False)
    w2_d = nc.declare_dram_parameter("w2", [R, D], f16, isOutput=False)
    out_d = nc.declare_dram_parameter("out", [T, D], f16, isOutput=True)

    n_groups = T // G

    with TileContext(nc) as tc:
        with (
            tc.tile_pool(name="const", bufs=1) as cpool,
            tc.tile_pool(name="xin", bufs=6) as xpool,
            tc.tile_pool(name="ysb", bufs=2) as ypool_sb,
            tc.tile_pool(name="gh", bufs=2) as ghpool,
            tc.tile_pool(name="zsb", bufs=2) as zpool_sb,
            tc.tile_pool(name="outp", bufs=4) as opool,
            tc.tile_pool(name="ps_y", bufs=1, space="PSUM") as psy,
            tc.tile_pool(name="ps_h", bufs=2, space="PSUM") as psh,
            tc.tile_pool(name="ps_z", bufs=1, space="PSUM") as psz,
            tc.tile_pool(name="ps_o", bufs=4, space="PSUM") as pso,
        ):
            # a_sb first (MM1's stationary), then the first x tile halves, and
            # only then the later-needed weights — so PE starts ~6us sooner.
            a_sb = cpool.tile([P, DC, R], f16)
            nc.sync.dma_start(out=a_sb[:], in_=a_d[:, :, :])
            xt0 = xpool.tile([P, DC, G], f16, tag="xt")
            nc.sync.dma_start(out=xt0[:, 0:6, :], in_=xT_d[:, 0:6, ts(0, G)])
            nc.sync.dma_start(out=xt0[:, 6:12, :], in_=xT_d[:, 6:12, ts(0, G)])
            m1_sb = cpool.tile([R, HC, P], f16)
            nc.sync.dma_start(out=m1_sb[:], in_=m1_d[:, :, :])
            m2_sb = cpool.tile([P, HC, R], f16)
            nc.sync.dma_start(out=m2_sb[:], in_=m2_d[:, :, :])
            w2_sb = cpool.tile([R, D], f16)
            nc.sync.dma_start(out=w2_sb[:], in_=w2_d[:, :])

            for g in range(n_groups):
                # input DMA per 512-token group, split in two halves so the
                # first MM1 chunks can start at half-arrival (HWDGE/sync ring)
                if g == 0:
                    xt = xt0
                else:
                    xt = xpool.tile([P, DC, G], f16, tag="xt")
                    nc.sync.dma_start(
                        out=xt[:, 0:6, :], in_=xT_d[:, 0:6, ts(g, G)]
                    )
                    nc.sync.dma_start(
                        out=xt[:, 6:12, :], in_=xT_d[:, 6:12, ts(g, G)]
                    )
                # ---- MM1: y^T[32, G] = sum_c A_c^T @ x^T_c ----
                y_ps = psy.tile([R, G], f32)
                for c in range(DC):
                    nc.tensor.matmul(
                        y_ps[:],
                        lhsT=a_sb[:, c, :],
                        rhs=xt[:, c, :],
                        start=(c == 0),
                        stop=(c == DC - 1),
                    )
                y_sb = ypool_sb.tile([R, G], f16)
                nc.vector.tensor_copy(out=y_sb[:], in_=y_ps[:])
                # ---- expand (kron rows) + gelu: gh[128p, G] per chunk ----
                gh = ghpool.tile([P, HC, G], f16)
                for m in range(HC):
                    h_ps = psh.tile([P, G], f32)
                    nc.tensor.matmul(
                        h_ps[:], lhsT=m1_sb[:, m, :], rhs=y_sb[:],
                        start=True, stop=True,
                    )
                    nc.scalar.activation(
                        out=gh[:, m, :], in_=h_ps[:], func=gelu_fn,
                    )
                # ---- contract l: z^T[32, G] = sum_m M2_m^T @ gh_m ----
                z_ps = psz.tile([R, G], f32)
                for m in range(HC):
                    nc.tensor.matmul(
                        z_ps[:], lhsT=m2_sb[:, m, :], rhs=gh[:, m, :],
                        start=(m == 0), stop=(m == HC - 1),
                    )
                z_sb = zpool_sb.tile([R, G], f16)
                nc.vector.tensor_copy(out=z_sb[:], in_=z_ps[:])
                # ---- MM2: out[128t, D] = z_blk @ W2, natural layout ----
                osb = opool.tile([P, n_b, D], f16)
                for b in range(n_b):
                    blk = g * n_b + b
                    for n in range(NO):
                        o_ps = pso.tile([P, 512], f32)
                        nc.tensor.matmul(
                            o_ps[:],
                            lhsT=z_sb[:, ts(b, P)],
                            rhs=w2_sb[:, ts(n, 512)],
                            start=True, stop=True,
                        )
                        # balance PSUM->SBUF casts across DVE and ACT
                        to_dve = (n == 0) or (n == 2 and blk % 8 < 5)
                        if to_dve:
                            nc.vector.tensor_copy(
                                out=osb[:, b, ts(n, 512)], in_=o_ps[:],
                            )
                        else:
                            nc.scalar.activation(
                                out=osb[:, b, ts(n, 512)], in_=o_ps[:],
                                func=AF.Copy,
                            )
                    # output DMA per block on the SWDGE (gpsimd) ring so it
                    # drains in parallel with the input stream's HWDGE ring
                    nc.gpsimd.dma_start(
                        out=out_d[ts(blk, P), :], in_=osb[:, b, :],
                    )
    nc.finalize()
    _dedup_ldweights(nc, mybir)
    return nc


def _dedup_ldweights(nc, mybir):
    """Remove InstLdweights whose stationary AP + tile params equal the
    immediately preceding tensor-engine weight load (with only matmuls in
    between). The PE array still holds those weights, so the reload is pure
    overhead (~90ns each, serialized with the matmul stream on TRN2).
    Dependencies of a removed load are migrated to the following matmul."""
    removed = 0
    for f in nc.m.functions:
        for blk in f.blocks:
            insts = list(blk.instructions)
            keep = []
            last_ldw_key = None
            pending = None  # ldw queued for possible removal
            for inst in insts:
                if isinstance(inst, mybir.InstLdweights):
                    hw = getattr(inst, "has_wait", None)
                    has_sync = bool(hw() if callable(hw) else hw)
                    key = (
                        str(inst.ins[0]),
                        str(getattr(inst, "tile_position", None)),
                        str(getattr(inst, "tile_size", None)),
                        str(getattr(inst, "perf_mode", None)),
                    )
                    if key == last_ldw_key and not has_sync:
                        pending = inst  # drop it; deps go to next matmul
                        removed += 1
                        continue
                    last_ldw_key = key
                    keep.append(inst)
                else:
                    # next inst (matmul or anything else): migrate the
                    # removed load's bookkeeping deps, keep the inst
                    if pending is not None:
                        inst.add_sync_dependencies_from(
                            pending.sync_dependency_set_copy()
                        )
                        inst.add_nosync_dependencies_from(
                            pending.nosync_dependency_set_copy()
                        )
                        pending = None
                    keep.append(inst)
            if pending is not None:
                keep.append(pending)
                removed -= 1
            if len(keep) != len(insts):
                blk.instructions = keep
    if os.environ.get("KRONY_DEBUG"):
        print(f"_dedup_ldweights: removed {removed}")
    return removed


def get_nc(dt_name=None):
    if dt_name is None:
        dt_name = os.environ.get("KRONY_DT", "f16")
    if dt_name not in _BUILT:
        _BUILT[dt_name] = _build(dt_name)
    return _BUILT[dt_name]


def _np_dt(dt_name):
    if dt_name == "f16":
        return np.float16
    import ml_dtypes
    return ml_dtypes.bfloat16


def _host_weights(c_fc_1, c_fc_2, c_proj_1, c_proj_2, np_dt):
    A = np.asarray(c_fc_1, np.float32)        # [1536, 32]
    Bv = np.asarray(c_fc_2, np.float32).reshape(L)     # [12]
    W2 = np.asarray(c_proj_1, np.float32)     # [32, 1536]
    B2 = np.asarray(c_proj_2, np.float32).reshape(L)   # [12]

    # d = p*12 + c  ->  a_dev[p, c, j] = A[p*12+c, j]
    a_dev = A.reshape(P, DC, R).astype(np_dt)
    # hidden index ph = j*12 + l ; chunk m holds ph in [m*128, (m+1)*128)
    ph = np.arange(H)
    j_of = ph // L
    l_of = ph % L
    # M1[ph, j] = B[l]*(j == j_of) ; lhsT for expand = M1^T [32, 384]
    m1t = np.zeros((R, H), np.float32)
    m1t[j_of, ph] = Bv[l_of]
    m1t_dev = m1t.reshape(R, HC, P).astype(np_dt)
    # M2[j, ph] = B2[l]*(j == j_of) (scaled); lhsT for contract = M2^T [384, 32]
    m2t = np.zeros((H, R), np.float32)
    m2t[ph, j_of] = B2[l_of] * OUT_SCALE
    m2t_dev = m2t.reshape(HC, P, R).transpose(1, 0, 2).copy()
    m2t_dev = np.ascontiguousarray(m2t_dev).astype(np_dt)
    w2_dev = W2.astype(np_dt)                 # [32, 1536]
    return a_dev, m1t_dev, m2t_dev, w2_dev


def _host_x(x, np_dt):
    """x [B, S, D] f32 -> per-core transposed [P, DC, T] fp16 (d = p*12+c)."""
    xf = np.asarray(x).reshape(N_CORES, T, P, DC).astype(np_dt)
    # [core, t, p, c] -> [core, p, c, t]
    return [np.ascontiguousarray(xf[i].transpose(1, 2, 0)) for i in range(N_CORES)]


def run_sharded(x, c_fc_1, c_fc_2, c_proj_1, c_proj_2, trace=False, tmpdir=None):
    from concourse.bass_utils import run_bass_kernel_spmd

    dt_name = os.environ.get("KRONY_DT", "f16")
    np_dt = _np_dt(dt_name)
    a_dev, m1t_dev, m2t_dev, w2_dev = _host_weights(
        c_fc_1, c_fc_2, c_proj_1, c_proj_2, np_dt
    )
    x_shards = _host_x(x, np_dt)
    in_maps = [
        {
            "xT": x_shards[i],
            "afc": a_dev,
            "m1t": m1t_dev,
            "m2t": m2t_dev,
            "w2": w2_dev,
        }
        for i in range(N_CORES)
    ]
    nc = get_nc(dt_name)
    res = run_bass_kernel_spmd(
        nc, in_maps, list(range(N_CORES)), trace=trace, tmpdir=tmpdir
    )
    outs = [res.results[i]["out"].astype(np.float32) for i in range(N_CORES)]
    full = np.concatenate(outs, axis=0) * np.float32(1.0 / OUT_SCALE)
    return full.reshape(B, S, D), res


def kernel(x, c_fc_1, c_fc_2, c_proj_1, c_proj_2):
    out, _ = run_sharded(x, c_fc_1, c_fc_2, c_proj_1, c_proj_2)
    return out.astype(np.float32)


# revision 18
# speedup vs baseline: 1.4008x; 1.2321x over previous
"""KronyMLP Trainium2 kernel — rank-32 Kronecker factorization, fp16, DMA-bound.

Math (per the reference):
    kr1 = kron(c_fc_1 [1536,32], c_fc_2 [1,12])     -> [1536, 384]
    kr2 = kron(c_proj_1 [32,1536], c_proj_2 [12,1]) -> [384, 1536]
    out = gelu(x @ kr1) @ kr2                        x: [16, 4096, 1536] f32

Key identity: kron structure makes both big matmuls rank-32:
    y  = x @ c_fc_1                    [T, 32]   (the only large contraction)
    h  = y expand-by c_fc_2            [T, 384]  h[t, j*12+l] = y[t,j]*B[l]
    g  = gelu(h)
    z  = g contract-by c_proj_2        [T, 32]   z[t,j] = sum_l g[t,j*12+l]*B2[l]
    out= z @ c_proj_1                  [T, 1536]
This cuts PE FLOPs ~12x vs the dense kr1/kr2 form; the kernel becomes
HBM-bound. fp16 I/O halves DMA traffic (tolerance is 2e-2). Output values
are ~1e-5 (subnormal in fp16), so a 2^8 scale is folded into the
contract-stage weights and divided out (exactly) on the host.

Per core (data-parallel over 8 cores, 8192 tokens each):
  - host pre-transposes x to [128, 12, 8192] fp16 (d = p*12 + c), so the
    device needs NO transposes: MM1 contracts d on partitions directly.
  - expand / contract stages are tiny matmuls against host-built
    kron-expansion matrices; gelu runs on ScalarE out of PSUM.
  - MM2 uses z^T token-blocks as stationary so the output lands in natural
    [token, d] layout for straight DMA out.
"""

import os
import numpy as np

B, S, D = 16, 4096, 1536
R = 32            # rank (c_fc_1 cols / c_proj_1 rows)
L = 12            # kron factor (c_fc_2 cols / c_proj_2 rows)
H = R * L         # 384 hidden
N_CORES = 8
T = (B // N_CORES) * S   # 8192 tokens per core
P = 128
DC = D // P       # 12 d chunks
HC = H // P       # 3 hidden chunks
G = 512           # tokens per matmul group (one x DMA, two halves)
OUT_SCALE = 256.0  # folded into M2 weights; divided out on host (exact)

_BUILT = {}


def _build(dt_name, T=T):
    import concourse.bacc as bacc
    import concourse.mybir as mybir
    from concourse.bass import ts
    from concourse.tile import TileContext

    f32 = mybir.dt.float32
    f16 = {"f16": mybir.dt.float16, "bf16": mybir.dt.bfloat16}[dt_name]
    AF = mybir.ActivationFunctionType
    # CoreSim doesn't implement Gelu; allow substituting Copy for sim runs.
    gelu_fn = AF.Copy if os.environ.get("KRONY_SIM_GELU") == "copy" else AF.Gelu

    n_b = G // P              # 4 token-blocks per group
    NO = D // 512             # 3 output column chunks

    nc = bacc.Bacc(None, target_bir_lowering=False, debug=False)
    xT_d = nc.declare_dram_parameter("xT", [P, DC, T], f16, isOutput=False)
    a_d = nc.declare_dram_parameter("afc", [P, DC, R], f16, isOutput=False)
    m1_d = nc.declare_dram_parameter("m1t", [R, HC, P], f16, isOutput=False)
    m2_d = nc.declare_dram_parameter("m2t", [P, HC, R], f16, isOutput=False)
    w2_d = nc.declare_dram_parameter("w2", [R, D], f16, isOutput=False)
    out_d = nc.declare_dram_parameter("out", [T, D], f16, isOutput=True)

    n_groups = T // G

    with TileContext(nc) as tc:
        with (
            tc.tile_pool(name="const", bufs=1) as cpool,
            tc.tile_pool(name="xin", bufs=8) as xpool,
            tc.tile_pool(name="ysb", bufs=2) as ypool_sb,
            tc.tile_pool(name="gh", bufs=2) as ghpool,
            tc.tile_pool(name="zsb", bufs=2) as zpool_sb,
            tc.tile_pool(name="outp", bufs=6) as opool,
            tc.tile_pool(name="ps_y", bufs=1, space="PSUM") as psy,
            tc.tile_pool(name="ps_h", bufs=2, space="PSUM") as psh,
            tc.tile_pool(name="ps_z", bufs=1, space="PSUM") as psz,
            tc.tile_pool(name="ps_o", bufs=4, space="PSUM") as pso,
        ):
            # a_sb first (MM1's stationary), then the first x tile in fine
            # slices (MM1 chunk c can start as soon as slice c lands), then
            # the later-needed weights — so PE starts ~6us sooner.
            a_sb = cpool.tile([P, DC, R], f16)
            nc.sync.dma_start(out=a_sb[:], in_=a_d[:, :, :])
            xt0 = xpool.tile([P, DC, G], f16, tag="xt")
            for h2 in range(4):
                nc.sync.dma_start(
                    out=xt0[:, 3 * h2 : 3 * (h2 + 1), :],
                    in_=xT_d[:, 3 * h2 : 3 * (h2 + 1), ts(0, G)],
                )
            m1_sb = cpool.tile([R, HC, P], f16)
            nc.sync.dma_start(out=m1_sb[:], in_=m1_d[:, :, :])
            m2_sb = cpool.tile([P, HC, R], f16)
            nc.sync.dma_start(out=m2_sb[:], in_=m2_d[:, :, :])
            w2_sb = cpool.tile([R, D], f16)
            nc.sync.dma_start(out=w2_sb[:], in_=w2_d[:, :])

            # PE pre-warm: ~16 tiny matmuls on a zeroed scratch run while the
            # first x slices are still in flight, so HAM reaches K=8/8 (and
            # the 2.4 GHz clock) before real work arrives.
            warm_sb = cpool.tile([P, 64], f16)
            nc.vector.memset(warm_sb[:], 0)
            warm_ps = pso.tile([R, 64], f32, tag="o_ps", name="warm_ps")
            for _ in range(16):
                nc.tensor.matmul(
                    warm_ps[:], lhsT=warm_sb[:, 0:R], rhs=warm_sb[:, 0:64],
                    start=True, stop=True,
                )

            for g in range(n_groups):
                # input DMA per 512-token group, split in two halves so the
                # first MM1 chunks can start at half-arrival (HWDGE/sync ring)
                if g == 0:
                    xt = xt0
                else:
                    xt = xpool.tile([P, DC, G], f16, tag="xt")
                    nc.sync.dma_start(
                        out=xt[:, 0:6, :], in_=xT_d[:, 0:6, ts(g, G)]
                    )
                    nc.sync.dma_start(
                        out=xt[:, 6:12, :], in_=xT_d[:, 6:12, ts(g, G)]
                    )
                # ---- MM1: y^T[32, G] = sum_c A_c^T @ x^T_c ----
                y_ps = psy.tile([R, G], f32)
                for c in range(DC):
                    nc.tensor.matmul(
                        y_ps[:],
                        lhsT=a_sb[:, c, :],
                        rhs=xt[:, c, :],
                        start=(c == 0),
                        stop=(c == DC - 1),
                    )
                y_sb = ypool_sb.tile([R, G], f16)
                nc.vector.tensor_copy(out=y_sb[:], in_=y_ps[:])
                # ---- expand (kron rows) + gelu: gh[128p, G] per chunk ----
                gh = ghpool.tile([P, HC, G], f16)
                for m in range(HC):
                    h_ps = psh.tile([P, G], f32)
                    nc.tensor.matmul(
                        h_ps[:], lhsT=m1_sb[:, m, :], rhs=y_sb[:],
                        start=True, stop=True,
                    )
                    nc.scalar.activation(
                        out=gh[:, m, :], in_=h_ps[:], func=gelu_fn,
                    )
                # ---- contract l: z^T[32, G] = sum_m M2_m^T @ gh_m ----
                z_ps = psz.tile([R, G], f32)
                for m in range(HC):
                    nc.tensor.matmul(
                        z_ps[:], lhsT=m2_sb[:, m, :], rhs=gh[:, m, :],
                        start=(m == 0), stop=(m == HC - 1),
                    )
                z_sb = zpool_sb.tile([R, G], f16)
                nc.vector.tensor_copy(out=z_sb[:], in_=z_ps[:])
                # ---- MM2: out[128t, D] = z_blk @ W2, natural layout ----
                osb = opool.tile([P, n_b, D], f16)
                for b in range(n_b):
                    blk = g * n_b + b
                    for n in range(NO):
                        o_ps = pso.tile([P, 512], f32)
                        nc.tensor.matmul(
                            o_ps[:],
                            lhsT=z_sb[:, ts(b, P)],
                            rhs=w2_sb[:, ts(n, 512)],
                            start=True, stop=True,
                        )
                        # balance PSUM->SBUF casts across DVE and ACT
                        to_dve = (n == 0) or (n == 2 and blk % 8 < 5)
                        if to_dve:
                            nc.vector.tensor_copy(
                                out=osb[:, b, ts(n, 512)], in_=o_ps[:],
                            )
                        else:
                            nc.scalar.activation(
                                out=osb[:, b, ts(n, 512)], in_=o_ps[:],
                                func=AF.Copy,
                            )
                    # output DMA per block on the SWDGE (gpsimd) ring so it
                    # drains in parallel with the input stream's HWDGE ring
                    nc.gpsimd.dma_start(
                        out=out_d[ts(blk, P), :], in_=osb[:, b, :],
                    )
    nc.finalize()
    _dedup_ldweights(nc, mybir)
    return nc


def _dedup_ldweights(nc, mybir):
    """Remove InstLdweights whose stationary AP + tile params equal the
    immediately preceding tensor-engine weight load (with only matmuls in
    between). The PE array still holds those weights, so the reload is pure
    overhead (~90ns each, serialized with the matmul stream on TRN2).
    Dependencies of a removed load are migrated to the following matmul."""
    removed = 0
    for f in nc.m.functions:
        for blk in f.blocks:
            insts = list(blk.instructions)
            keep = []
            last_ldw_key = None
            pending = None  # ldw queued for possible removal
            for inst in insts:
                if isinstance(inst, mybir.InstLdweights):
                    hw = getattr(inst, "has_wait", None)
                    has_sync = bool(hw() if callable(hw) else hw)
                    key = (
                        str(inst.ins[0]),
                        str(getattr(inst, "tile_position", None)),
                        str(getattr(inst, "tile_size", None)),
                        str(getattr(inst, "perf_mode", None)),
                    )
                    if key == last_ldw_key and not has_sync:
                        pending = inst  # drop it; deps go to next matmul
                        removed += 1
                        continue
                    last_ldw_key = key
                    keep.append(inst)
                else:
                    # next inst (matmul or anything else): migrate the
                    # removed load's bookkeeping deps, keep the inst
                    if pending is not None:
                        inst.add_sync_dependencies_from(
                            pending.sync_dependency_set_copy()
                        )
                        inst.add_nosync_dependencies_from(
                            pending.nosync_dependency_set_copy()
                        )
                        pending = None
                    keep.append(inst)
            if pending is not None:
                keep.append(pending)
                removed -= 1
            if len(keep) != len(insts):
                blk.instructions = keep
    if os.environ.get("KRONY_DEBUG"):
        print(f"_dedup_ldweights: removed {removed}")
    return removed


def get_nc(dt_name=None):
    if dt_name is None:
        dt_name = os.environ.get("KRONY_DT", "f16")
    if dt_name not in _BUILT:
        _BUILT[dt_name] = _build(dt_name)
    return _BUILT[dt_name]


def _np_dt(dt_name):
    if dt_name == "f16":
        return np.float16
    import ml_dtypes
    return ml_dtypes.bfloat16


def _host_weights(c_fc_1, c_fc_2, c_proj_1, c_proj_2, np_dt):
    A = np.asarray(c_fc_1, np.float32)        # [1536, 32]
    Bv = np.asarray(c_fc_2, np.float32).reshape(L)     # [12]
    W2 = np.asarray(c_proj_1, np.float32)     # [32, 1536]
    B2 = np.asarray(c_proj_2, np.float32).reshape(L)   # [12]

    # d = p*12 + c  ->  a_dev[p, c, j] = A[p*12+c, j]
    a_dev = A.reshape(P, DC, R).astype(np_dt)
    # hidden index ph = j*12 + l ; chunk m holds ph in [m*128, (m+1)*128)
    ph = np.arange(H)
    j_of = ph // L
    l_of = ph % L
    # M1[ph, j] = B[l]*(j == j_of) ; lhsT for expand = M1^T [32, 384]
    m1t = np.zeros((R, H), np.float32)
    m1t[j_of, ph] = Bv[l_of]
    m1t_dev = m1t.reshape(R, HC, P).astype(np_dt)
    # M2[j, ph] = B2[l]*(j == j_of) (scaled); lhsT for contract = M2^T [384, 32]
    m2t = np.zeros((H, R), np.float32)
    m2t[ph, j_of] = B2[l_of] * OUT_SCALE
    m2t_dev = m2t.reshape(HC, P, R).transpose(1, 0, 2).copy()
    m2t_dev = np.ascontiguousarray(m2t_dev).astype(np_dt)
    w2_dev = W2.astype(np_dt)                 # [32, 1536]
    return a_dev, m1t_dev, m2t_dev, w2_dev


def _host_x(x, np_dt):
    """x [B, S, D] f32 -> per-core transposed [P, DC, T] fp16 (d = p*12+c)."""
    xf = np.asarray(x).reshape(N_CORES, T, P, DC).astype(np_dt)
    # [core, t, p, c] -> [core, p, c, t]
    return [np.ascontiguousarray(xf[i].transpose(1, 2, 0)) for i in range(N_CORES)]


def run_sharded(x, c_fc_1, c_fc_2, c_proj_1, c_proj_2, trace=False, tmpdir=None):
    from concourse.bass_utils import run_bass_kernel_spmd

    dt_name = os.environ.get("KRONY_DT", "f16")
    np_dt = _np_dt(dt_name)
    a_dev, m1t_dev, m2t_dev, w2_dev = _host_weights(
        c_fc_1, c_fc_2, c_proj_1, c_proj_2, np_dt
    )
    x_shards = _host_x(x, np_dt)
    in_maps = [
        {
            "xT": x_shards[i],
            "afc": a_dev,
            "m1t": m1t_dev,
            "m2t": m2t_dev,
            "w2": w2_dev,
        }
        for i in range(N_CORES)
    ]
    nc = get_nc(dt_name)
    res = run_bass_kernel_spmd(
        nc, in_maps, list(range(N_CORES)), trace=trace, tmpdir=tmpdir
    )
    outs = [res.results[i]["out"].astype(np.float32) for i in range(N_CORES)]
    full = np.concatenate(outs, axis=0) * np.float32(1.0 / OUT_SCALE)
    return full.reshape(B, S, D), res


def kernel(x, c_fc_1, c_fc_2, c_proj_1, c_proj_2):
    out, _ = run_sharded(x, c_fc_1, c_fc_2, c_proj_1, c_proj_2)
    return out.astype(np.float32)
